# revision 7
# baseline (speedup 1.0000x reference)
"""Softmax-attention pooling kernel for Trainium2 (8 NeuronCores).

Reference computation (N=1,000,000, D=128):
    scores = (x @ W.T + b).reshape(1, -1)     # [1, N]
    weight = softmax(scores, axis=1)          # over all N
    out    = weight @ x                       # [1, D]

Strategy:
  - Shard x row-wise across 8 cores (125,000 rows each, zero-padded to
    125,440 = 980 tiles of 128 rows so every matmul group has 4 tiles).
  - Per core, single pass over x (64 MB -> DMA-bound ~180us):
      * scores per 128x128 tile via one fused VectorE
        tensor_tensor_reduce (x_tile * W_row, reduced over D) -> [128,1]
      * e = exp(scores) on ScalarE, with accum_out giving per-round sum(e)
      * unnormalized weighted sum via TensorE: lhsT = e columns [128,4],
        rhs = 4 x-tiles [128,512] (float32r, 1 cyc/row), accumulated
        block-diagonally in one PSUM bank across the whole kernel
  - b is ignored: softmax is invariant to a constant shift.
  - Host combines per-core partials exactly in float64:
        out = sum_c acc_c / (sum_c esum_c - n_pad)
    (each zero-padded row contributes exactly exp(0)=1 to esum and 0 to acc)
"""

import sys

if "/opt/trn_rl_repo" not in sys.path:
    sys.path.insert(0, "/opt/trn_rl_repo")

import numpy as np

import concourse.bass as bass
import concourse.tile as tile
from concourse import mybir
from concourse.vector_clock import ScopedClock
from concourse.bass_utils import run_bass_kernel_spmd
from concourse.dve_ops import TENSOR_TENSOR_REDUCE

N = 1_000_000
D = 128
NCORES = 8
ROWS_PER_CORE = N // NCORES          # 125,000
TILES = 980                          # 980*128 = 125,440 padded rows per core
PAD_ROWS = TILES * 128 - ROWS_PER_CORE  # 440
PADDED_ROWS = TILES * 128            # 125,440
ROUNDS = [64] * 15 + [20]            # tiles per round; sum = 980
NROUNDS = len(ROUNDS)

F32 = mybir.dt.float32
F32R = mybir.dt.float32r

_MAX_WAITS = 1  # this walrus build allows one semaphore wait per CTRL inst


def _patched_drain_and_barrier(self, tick_clock, wait_clock):
    """TileContext exit drain, with sem waits split one-per-instruction.

    The stock exit path attaches every outstanding proc's semaphore wait to a
    single SP Drain, which this walrus rejects ("Too many sync wait
    commands").  Overflow waits are moved to nofuse SP nops that run before
    the barrier/sem-clear, preserving the join semantics.
    """
    nc = self.nc
    drain_inst = nc.sync.drain()
    wait_clock.add_sem_waits(
        drain_inst.ins, ScopedClock({None: tick_clock.global_clock})
    )
    ins = drain_inst.ins
    si = ins.sync_info
    waits = list(si.on_wait or []) if si is not None else []
    if len(waits) > _MAX_WAITS:
        si.on_wait = waits[:_MAX_WAITS]
        ins.sync_info = si
        for i in range(_MAX_WAITS, len(waits), _MAX_WAITS):
            nop_inst = nc.sync.nop(nofuse=True)
            nsi = nop_inst.ins.sync_info or mybir.SyncInfo(on_wait=[], on_update=[])
            nsi.on_wait = waits[i : i + _MAX_WAITS]
            nop_inst.ins.sync_info = nsi
    nc.all_engine_barrier()
    popped = nc._tile_sem_poison_stack.pop()
    assert popped is self._sem_poison
    nc.clear_and_free_semaphores(list(self.sems.allocated().values()))
    nc.all_engine_barrier()


tile.TileContext._drain_and_barrier = _patched_drain_and_barrier


def _build_program() -> bass.Bass:
    nc = bass.Bass("TRN2", target_bir_lowering=False, debug=False, num_devices=NCORES)

    x_in = nc.dram_tensor("x", [PADDED_ROWS, D], F32, kind="ExternalInput").ap()
    w_in = nc.dram_tensor("w", [1, D], F32, kind="ExternalInput").ap()
    acc_out = nc.dram_tensor("acc", [4, 4 * D], F32, kind="ExternalOutput").ap()
    esum_out = nc.dram_tensor("esums", [128, NROUNDS], F32, kind="ExternalOutput").ap()

    with tile.TileContext(nc) as tc:
        with (
            tc.tile_pool(name="singles", bufs=1) as singles,
            tc.tile_pool(name="xc", bufs=3) as xpool,
            tc.tile_pool(name="sc", bufs=3) as spool,
            tc.tile_pool(name="ec", bufs=3) as epool,
            tc.tile_pool(name="scratch", bufs=2) as scratch,
            tc.tile_pool(name="psum", bufs=1, space="PSUM") as psum,
        ):
            # W row broadcast to all 128 partitions, loaded once.
            wb = singles.tile([128, D], F32)
            nc.sync.dma_start(out=wb[:], in_=w_in.to_broadcast([128, D]))
            # Per-round sum(exp(scores)) columns; DMA'd out at the end.
            esums = singles.tile([128, NROUNDS], F32)
            # Persistent PSUM accumulator (one bank): block-diagonal partials.
            accp = psum.tile([4, 4 * D], F32)

            n_groups_total = sum(r // 4 for r in ROUNDS)
            group_idx = 0
            r0 = 0
            for ridx, R in enumerate(ROUNDS):
                # Linear chunk: partition p holds R consecutive rows
                # (rows r0*128 + p*R .. +R-1), fully contiguous DMA.
                src = x_in[r0 * 128 : (r0 + R) * 128, :].rearrange(
                    "(p k) d -> p (k d)", p=128
                )
                # Tile dtype is float32r (bit-identical to fp32; the PE rounds
                # to TF32 internally) so the BIR verifier accepts it as a
                # rounded fp32r matmul input. TTR reads it as plain fp32.
                xc = xpool.tile([128, R * D], F32R, tag="xc")
                nc.sync.dma_start(out=xc[:], in_=src.bitcast(F32R))

                # scores[p, k] = sum_d x[row(p,k), d] * W[d]
                # One fused custom-DVE op per tile: out = in0*in1*s1,
                # accum_out = s0 + sum(out) (registered TENSOR_TENSOR_REDUCE).
                sc = spool.tile([128, R], F32, tag="sc")
                for k in range(R):
                    prod = scratch.tile([128, D], F32, tag="prod")
                    nc.vector._custom_dve(
                        TENSOR_TENSOR_REDUCE,
                        out=prod[:],
                        in0=xc[:, k * D : (k + 1) * D].bitcast(F32),
                        in1=wb[:],
                        s0=0.0,
                        s1=1.0,
                        accum_out=sc[:, k : k + 1],
                    )

                # e = exp(scores); accum_out = per-partition sum over round
                ec = epool.tile([128, R], F32, tag="ec")
                nc.scalar.activation(
                    out=ec[:],
                    in_=sc[:],
                    func=mybir.ActivationFunctionType.Exp,
                    bias=0.0,
                    scale=1.0,
                    accum_out=esums[:, ridx : ridx + 1],
                )
                # Rounded fp32r view of e for the PE (tiny DVE copy).
                ecr = epool.tile([128, R], F32R, tag="ecr")
                nc.vector.tensor_copy(out=ecr[:], in_=ec[:])

                # Weighted partial sums: groups of 4 tiles -> [4, 512] PSUM
                for g in range(0, R, 4):
                    nc.tensor.matmul(
                        out=accp[:],
                        lhsT=ecr[:, g : g + 4],
                        rhs=xc[:, g * D : (g + 4) * D],
                        start=(group_idx == 0),
                        stop=(group_idx == n_groups_total - 1),
                    )
                    group_idx += 1
                r0 += R

            # Epilogue: PSUM -> SBUF -> DRAM, esums -> DRAM
            acc_sb = singles.tile([4, 4 * D], F32)
            nc.scalar.activation(
                out=acc_sb[:],
                in_=accp[:],
                func=mybir.ActivationFunctionType.Copy,
            )
            nc.sync.dma_start(out=acc_out[:], in_=acc_sb[:])
            nc.sync.dma_start(out=esum_out[:], in_=esums[:])

    # Populate .instr bytes for InstISA subclasses (custom DVE op); raw Bass
    # skips this pass and walrus rejects empty encodings ("ISA wrong length").
    mybir.codegen_inst_isa_subclasses(nc)
    _split_multiwait_instructions(nc)
    return nc


def _split_multiwait_instructions(nc: bass.Bass, max_waits: int = _MAX_WAITS):
    """Hoist excess semaphore waits onto same-engine nops inserted before the
    instruction — this walrus build allows only one sync wait per instruction.
    """
    import bass_rust

    for func in nc.m.functions:
        for block in func.blocks:
            insts = list(block.instructions)
            out = []
            changed = False
            for inst in insts:
                si = inst.sync_info
                waits = list(si.on_wait or []) if si is not None else []
                if len(waits) > max_waits:
                    extra, keep = waits[:-max_waits], waits[-max_waits:]
                    for i in range(0, len(extra), max_waits):
                        nop = bass_rust.InstNoOp(
                            name=nc.get_next_instruction_name(),
                            engine=inst.engine,
                            ins=[],
                            outs=[],
                        )
                        nop.sync_info = mybir.SyncInfo(
                            on_wait=extra[i : i + max_waits], on_update=[]
                        )
                        nc.inst_map[nop.name] = nop
                        out.append(nop)
                    si.on_wait = keep
                    inst.sync_info = si
                    changed = True
                out.append(inst)
            if changed:
                block.instructions[:] = out


_NC_CACHE = None


def _get_program():
    global _NC_CACHE
    if _NC_CACHE is None:
        _NC_CACHE = _build_program()
    return _NC_CACHE


def _run(in_maps, trace=False, trace_kwargs=None):
    nc = _get_program()
    kw = {}
    if trace:
        kw["trace"] = True
        if trace_kwargs:
            kw["trace_kwargs"] = trace_kwargs
    return run_bass_kernel_spmd(nc, in_maps, list(range(NCORES)), **kw)


def _shard_inputs(x: np.ndarray, W: np.ndarray):
    """Pad + shard x row-wise; returns per-core input maps."""
    x = np.ascontiguousarray(x, dtype=np.float32)
    W = np.ascontiguousarray(W, dtype=np.float32).reshape(1, D)
    in_maps = []
    for c in range(NCORES):
        shard = np.zeros((PADDED_ROWS, D), dtype=np.float32)
        shard[:ROWS_PER_CORE] = x[c * ROWS_PER_CORE : (c + 1) * ROWS_PER_CORE]
        in_maps.append({"x": shard, "w": W})
    return in_maps


def _combine(results) -> np.ndarray:
    """Exact distributed-softmax combine in float64."""
    num = np.zeros(D, dtype=np.float64)
    den = 0.0
    for c in range(NCORES):
        acc = results[c]["acc"].astype(np.float64)  # [4, 512]
        esum = results[c]["esums"].astype(np.float64).sum()
        # Valid data is block-diagonal: row j holds cols j*128:(j+1)*128
        for j in range(4):
            num += acc[j, j * D : (j + 1) * D]
        den += esum - PAD_ROWS  # each padded row contributed exp(0) = 1
    return (num / den).astype(np.float32).reshape(1, D)


def kernel(x: np.ndarray, W: np.ndarray, b: np.ndarray) -> np.ndarray:
    # b shifts every score equally; softmax is invariant to it.
    del b
    res = _run(_shard_inputs(np.asarray(x), np.asarray(W)))
    return _combine(res.results)


if __name__ == "__main__":
    # Tiny self-check against numpy on random data
    rng = np.random.default_rng(0)
    x = rng.standard_normal((N, D), dtype=np.float32)
    W = (rng.standard_normal((1, D), dtype=np.float32) / np.sqrt(D)).astype(np.float32)
    b = np.zeros(1, dtype=np.float32)
    out = kernel(x, W, b)
    s = (x.astype(np.float64) @ W.astype(np.float64).T).reshape(-1)
    w_ = np.exp(s - s.max())
    w_ /= w_.sum()
    ref = (w_ @ x.astype(np.float64)).reshape(1, D)
    err = np.abs(out - ref).max() / np.abs(ref).max()
    print("max-rel-to-scale error vs fp64 numpy:", err)


# revision 9
# speedup vs baseline: 1.0308x; 1.0308x over previous
"""Softmax-attention pooling kernel for Trainium2 (8 NeuronCores).

Reference computation (N=1,000,000, D=128):
    scores = (x @ W.T + b).reshape(1, -1)     # [1, N]
    weight = softmax(scores, axis=1)          # over all N
    out    = weight @ x                       # [1, D]

Strategy:
  - Shard x row-wise across 8 cores (125,000 rows each, zero-padded to
    125,440 = 980 tiles of 128 rows so every matmul group has 4 tiles).
  - Per core, single pass over x (64 MB -> DMA-bound ~180us):
      * scores per 128x128 tile via one fused VectorE
        tensor_tensor_reduce (x_tile * W_row, reduced over D) -> [128,1]
      * e = exp(scores) on ScalarE, with accum_out giving per-round sum(e)
      * unnormalized weighted sum via TensorE: lhsT = e columns [128,4],
        rhs = 4 x-tiles [128,512] (float32r, 1 cyc/row), accumulated
        block-diagonally in one PSUM bank across the whole kernel
  - b is ignored: softmax is invariant to a constant shift.
  - Host combines per-core partials exactly in float64:
        out = sum_c acc_c / (sum_c esum_c - n_pad)
    (each zero-padded row contributes exactly exp(0)=1 to esum and 0 to acc)
"""

import sys

if "/opt/trn_rl_repo" not in sys.path:
    sys.path.insert(0, "/opt/trn_rl_repo")

import numpy as np

import concourse.bass as bass
import concourse.tile as tile
from concourse import mybir
from concourse.vector_clock import ScopedClock
from concourse.bass_utils import run_bass_kernel_spmd
from concourse.dve_ops import TENSOR_TENSOR_REDUCE

N = 1_000_000
D = 128
NCORES = 8
ROWS_PER_CORE = N // NCORES          # 125,000
TILES = 980                          # 980*128 = 125,440 padded rows per core
PAD_ROWS = TILES * 128 - ROWS_PER_CORE  # 440
PADDED_ROWS = TILES * 128            # 125,440
ROUNDS = [16] * 61 + [4]             # tiles per round; sum = 980 (1MB DMAs)
NROUNDS = len(ROUNDS)

F32 = mybir.dt.float32
F32R = mybir.dt.float32r

_MAX_WAITS = 1  # this walrus build allows one semaphore wait per CTRL inst


def _patched_drain_and_barrier(self, tick_clock, wait_clock):
    """TileContext exit drain, with sem waits split one-per-instruction.

    The stock exit path attaches every outstanding proc's semaphore wait to a
    single SP Drain, which this walrus rejects ("Too many sync wait
    commands").  Overflow waits are moved to nofuse SP nops that run before
    the barrier/sem-clear, preserving the join semantics.
    """
    nc = self.nc
    drain_inst = nc.sync.drain()
    wait_clock.add_sem_waits(
        drain_inst.ins, ScopedClock({None: tick_clock.global_clock})
    )
    ins = drain_inst.ins
    si = ins.sync_info
    waits = list(si.on_wait or []) if si is not None else []
    if len(waits) > _MAX_WAITS:
        si.on_wait = waits[:_MAX_WAITS]
        ins.sync_info = si
        for i in range(_MAX_WAITS, len(waits), _MAX_WAITS):
            nop_inst = nc.sync.nop(nofuse=True)
            nsi = nop_inst.ins.sync_info or mybir.SyncInfo(on_wait=[], on_update=[])
            nsi.on_wait = waits[i : i + _MAX_WAITS]
            nop_inst.ins.sync_info = nsi
    nc.all_engine_barrier()
    popped = nc._tile_sem_poison_stack.pop()
    assert popped is self._sem_poison
    nc.clear_and_free_semaphores(list(self.sems.allocated().values()))
    nc.all_engine_barrier()


tile.TileContext._drain_and_barrier = _patched_drain_and_barrier


def _build_program() -> bass.Bass:
    nc = bass.Bass("TRN2", target_bir_lowering=False, debug=False, num_devices=NCORES)

    x_in = nc.dram_tensor("x", [PADDED_ROWS, D], F32, kind="ExternalInput").ap()
    w_in = nc.dram_tensor("w", [1, D], F32, kind="ExternalInput").ap()
    acc_out = nc.dram_tensor("acc", [4, 4 * D], F32, kind="ExternalOutput").ap()
    esum_out = nc.dram_tensor("esums", [128, NROUNDS], F32, kind="ExternalOutput").ap()

    with tile.TileContext(nc) as tc:
        with (
            tc.tile_pool(name="singles", bufs=1) as singles,
            tc.tile_pool(name="xc", bufs=8) as xpool,
            tc.tile_pool(name="sc", bufs=4) as spool,
            tc.tile_pool(name="ec", bufs=4) as epool,
            tc.tile_pool(name="scratch", bufs=4) as scratch,
            tc.tile_pool(name="psum", bufs=1, space="PSUM") as psum,
        ):
            # W row broadcast to all 128 partitions, loaded once.
            wb = singles.tile([128, D], F32)
            nc.sync.dma_start(out=wb[:], in_=w_in.to_broadcast([128, D]))
            # Per-round sum(exp(scores)) columns; DMA'd out at the end.
            esums = singles.tile([128, NROUNDS], F32)
            # Persistent PSUM accumulator (one bank): block-diagonal partials.
            accp = psum.tile([4, 4 * D], F32)

            n_groups_total = sum(r // 4 for r in ROUNDS)
            group_idx = 0
            r0 = 0
            for ridx, R in enumerate(ROUNDS):
                # Linear chunk: partition p holds R consecutive rows
                # (rows r0*128 + p*R .. +R-1), fully contiguous DMA.
                src = x_in[r0 * 128 : (r0 + R) * 128, :].rearrange(
                    "(p k) d -> p (k d)", p=128
                )
                # Tile dtype is float32r (bit-identical to fp32; the PE rounds
                # to TF32 internally) so the BIR verifier accepts it as a
                # rounded fp32r matmul input. TTR reads it as plain fp32.
                xc = xpool.tile([128, R * D], F32R, tag="xc")
                nc.sync.dma_start(out=xc[:], in_=src.bitcast(F32R))

                # scores[p, k] = sum_d x[row(p,k), d] * W[d]
                # One fused custom-DVE op per tile: out = in0*in1*s1,
                # accum_out = s0 + sum(out) (registered TENSOR_TENSOR_REDUCE).
                sc = spool.tile([128, R], F32, tag="sc")
                for k in range(R):
                    prod = scratch.tile([128, D], F32, tag="prod")
                    nc.vector._custom_dve(
                        TENSOR_TENSOR_REDUCE,
                        out=prod[:],
                        in0=xc[:, k * D : (k + 1) * D].bitcast(F32),
                        in1=wb[:],
                        s0=0.0,
                        s1=1.0,
                        accum_out=sc[:, k : k + 1],
                    )

                # e = exp(scores); accum_out = per-partition sum over round
                ec = epool.tile([128, R], F32, tag="ec")
                nc.scalar.activation(
                    out=ec[:],
                    in_=sc[:],
                    func=mybir.ActivationFunctionType.Exp,
                    bias=0.0,
                    scale=1.0,
                    accum_out=esums[:, ridx : ridx + 1],
                )
                # Rounded fp32r view of e for the PE (tiny DVE copy).
                ecr = epool.tile([128, R], F32R, tag="ecr")
                nc.vector.tensor_copy(out=ecr[:], in_=ec[:])

                # Weighted partial sums: groups of 4 tiles -> [4, 512] PSUM
                for g in range(0, R, 4):
                    nc.tensor.matmul(
                        out=accp[:],
                        lhsT=ecr[:, g : g + 4],
                        rhs=xc[:, g * D : (g + 4) * D],
                        start=(group_idx == 0),
                        stop=(group_idx == n_groups_total - 1),
                    )
                    group_idx += 1
                r0 += R

            # Epilogue: PSUM -> SBUF -> DRAM, esums -> DRAM
            acc_sb = singles.tile([4, 4 * D], F32)
            nc.scalar.activation(
                out=acc_sb[:],
                in_=accp[:],
                func=mybir.ActivationFunctionType.Copy,
            )
            nc.sync.dma_start(out=acc_out[:], in_=acc_sb[:])
            nc.sync.dma_start(out=esum_out[:], in_=esums[:])

    # Populate .instr bytes for InstISA subclasses (custom DVE op); raw Bass
    # skips this pass and walrus rejects empty encodings ("ISA wrong length").
    mybir.codegen_inst_isa_subclasses(nc)
    _split_multiwait_instructions(nc)
    return nc


def _split_multiwait_instructions(nc: bass.Bass, max_waits: int = _MAX_WAITS):
    """Hoist excess semaphore waits onto same-engine nops inserted before the
    instruction — this walrus build allows only one sync wait per instruction.
    """
    import bass_rust

    for func in nc.m.functions:
        for block in func.blocks:
            insts = list(block.instructions)
            out = []
            changed = False
            for inst in insts:
                si = inst.sync_info
                waits = list(si.on_wait or []) if si is not None else []
                if len(waits) > max_waits:
                    extra, keep = waits[:-max_waits], waits[-max_waits:]
                    for i in range(0, len(extra), max_waits):
                        nop = bass_rust.InstNoOp(
                            name=nc.get_next_instruction_name(),
                            engine=inst.engine,
                            ins=[],
                            outs=[],
                        )
                        nop.sync_info = mybir.SyncInfo(
                            on_wait=extra[i : i + max_waits], on_update=[]
                        )
                        nc.inst_map[nop.name] = nop
                        out.append(nop)
                    si.on_wait = keep
                    inst.sync_info = si
                    changed = True
                out.append(inst)
            if changed:
                block.instructions[:] = out


_NC_CACHE = None


def _get_program():
    global _NC_CACHE
    if _NC_CACHE is None:
        _NC_CACHE = _build_program()
    return _NC_CACHE


def _run(in_maps, trace=False, trace_kwargs=None):
    nc = _get_program()
    kw = {}
    if trace:
        kw["trace"] = True
        if trace_kwargs:
            kw["trace_kwargs"] = trace_kwargs
    return run_bass_kernel_spmd(nc, in_maps, list(range(NCORES)), **kw)


def _shard_inputs(x: np.ndarray, W: np.ndarray):
    """Pad + shard x row-wise; returns per-core input maps."""
    x = np.ascontiguousarray(x, dtype=np.float32)
    W = np.ascontiguousarray(W, dtype=np.float32).reshape(1, D)
    in_maps = []
    for c in range(NCORES):
        shard = np.zeros((PADDED_ROWS, D), dtype=np.float32)
        shard[:ROWS_PER_CORE] = x[c * ROWS_PER_CORE : (c + 1) * ROWS_PER_CORE]
        in_maps.append({"x": shard, "w": W})
    return in_maps


def _combine(results) -> np.ndarray:
    """Exact distributed-softmax combine in float64."""
    num = np.zeros(D, dtype=np.float64)
    den = 0.0
    for c in range(NCORES):
        acc = results[c]["acc"].astype(np.float64)  # [4, 512]
        esum = results[c]["esums"].astype(np.float64).sum()
        # Valid data is block-diagonal: row j holds cols j*128:(j+1)*128
        for j in range(4):
            num += acc[j, j * D : (j + 1) * D]
        den += esum - PAD_ROWS  # each padded row contributed exp(0) = 1
    return (num / den).astype(np.float32).reshape(1, D)


def kernel(x: np.ndarray, W: np.ndarray, b: np.ndarray) -> np.ndarray:
    # b shifts every score equally; softmax is invariant to it.
    del b
    res = _run(_shard_inputs(np.asarray(x), np.asarray(W)))
    return _combine(res.results)


if __name__ == "__main__":
    # Tiny self-check against numpy on random data
    rng = np.random.default_rng(0)
    x = rng.standard_normal((N, D), dtype=np.float32)
    W = (rng.standard_normal((1, D), dtype=np.float32) / np.sqrt(D)).astype(np.float32)
    b = np.zeros(1, dtype=np.float32)
    out = kernel(x, W, b)
    s = (x.astype(np.float64) @ W.astype(np.float64).T).reshape(-1)
    w_ = np.exp(s - s.max())
    w_ /= w_.sum()
    ref = (w_ @ x.astype(np.float64)).reshape(1, D)
    err = np.abs(out - ref).max() / np.abs(ref).max()
    print("max-rel-to-scale error vs fp64 numpy:", err)
